# revision 24
# baseline (speedup 1.0000x reference)
"""Multi-head attention (B=2, S=2048, H=16, D=64) on 8 Trainium2 NeuronCores.

Head-parallel tensor parallelism: core c owns heads {2c, 2c+1} (a 128-dim
slice of the model dim): column-parallel QKV projections and local causal
attention for its 2 heads, then an AllToAll of bf16 context vectors (one
half-batch at a time, pipelined behind attention) and a full-width Wo
projection for this core's own disjoint 128-token output slices.

Differences from the earlier revision of this kernel, all aimed at the
Tensor engine (the measured bottleneck) and the serial head/tail:

* x is loaded in 8 per-token-tile tiles instead of one monolithic tile, and
  the QKV projection of tile t is interleaved with attention on query group
  t-1, so compute starts ~6 us in instead of waiting ~50 us for all of x.
* Attention-times-V keeps V (with a prepended ones column) as the 65-column
  stationary operand and streams the exp tile: one matmul per key block
  (N up to 512) instead of four LDWEIGHTS-bound N=66 matmuls, and the
  context comes out already transposed ([dims, tokens]) so the per-qt PE
  transposes of ctx are gone entirely.
* Scores use tile_position row pairs: each head is a K=64 matmul on its own
  row-group half of the PE array, so the two heads' score matmuls run
  concurrently instead of zero-padding K to 128.
* The ones column is first in the V operand, so the softmax denominator
  lands on PSUM partition 0: reciprocal on the DVE, partition_broadcast on
  the (otherwise idle) GpSimd engine, and one fused DVE multiply normalize
  the context without any extra PE work.
* exp is one ACT instruction per key block covering both heads (PSUM scores
  tile is [128, 2, 512]), halving the per-instruction ACT overhead.
* A tiny warm-up AllToAll is issued during the load phase so the first real
  collective doesn't pay the ~23 us first-call setup on the critical path.
"""

import sys

sys.path.insert(0, "/opt/trn_rl_repo")

import ml_dtypes
import numpy as np

import concourse.bass as bass
import concourse.tile as tile
from concourse import bacc, mybir
from concourse.bass_utils import run_bass_kernel_spmd

N_CORES = 8
B, S, H, D = 2, 2048, 16, 64
E = H * D            # 1024
T = B * S            # 4096 tokens
DPC = 128            # dims (2 heads) per core
NKC = E // 128       # 8 contraction chunks for the projections
NTT = T // 512       # 8 token tiles of 512
SB = S // 128        # 16 key blocks per batch
PH = S // 2 // N_CORES  # 128 tokens per core per half-batch

F32 = mybir.dt.float32
BF16 = mybir.dt.bfloat16
AFT = mybir.ActivationFunctionType


def build_program(debug_taps=False):
    nc = bacc.Bacc("TRN2", target_bir_lowering=False, debug=False,
                   num_devices=N_CORES)
    if debug_taps:
        dbg_qk = nc.dram_tensor("dbg_qk", [128, 2, T], BF16,
                                kind="ExternalOutput").ap()
        dbg_vn = nc.dram_tensor("dbg_vn", [128, T // 128, 130], BF16,
                                kind="ExternalOutput").ap()
        dbg_e = nc.dram_tensor("dbg_e", [128, 2, 512], BF16,
                               kind="ExternalOutput").ap()
        dbg_cn = nc.dram_tensor("dbg_cn", [128, 2, 512], F32,
                                kind="ExternalOutput").ap()
        dbg_bc = nc.dram_tensor("dbg_bc", [64, 2, 512], F32,
                                kind="ExternalOutput").ap()
        dbg_rr = nc.dram_tensor("dbg_rr", [1, 2, 512], F32,
                                kind="ExternalOutput").ap()
        dbg_ctx = nc.dram_tensor("dbg_ctx", [64, 2, T], BF16,
                                 kind="ExternalOutput").ap()
        dbg_cg = nc.dram_tensor("dbg_cg", [128, NKC, PH], BF16,
                                kind="ExternalOutput").ap()

    xT = nc.dram_tensor("xT", [E, T], BF16, kind="ExternalInput").ap()
    wqT = nc.dram_tensor("wqT", [E, DPC], BF16, kind="ExternalInput").ap()
    wkT = nc.dram_tensor("wkT", [E, DPC], BF16, kind="ExternalInput").ap()
    wvT = nc.dram_tensor("wvT", [E, DPC], BF16, kind="ExternalInput").ap()
    woT = nc.dram_tensor("woT", [E, E], BF16, kind="ExternalInput").ap()
    bq = nc.dram_tensor("bq", [DPC, 1], F32, kind="ExternalInput").ap()
    bk = nc.dram_tensor("bk", [DPC, 1], F32, kind="ExternalInput").ap()
    bv = nc.dram_tensor("bv", [DPC, 1], F32, kind="ExternalInput").ap()
    bo = nc.dram_tensor("bo", [E], F32, kind="ExternalInput").ap()
    # single 128x128 lower-triangular (k_local <= q_local) mask
    tri = nc.dram_tensor("tri", [128, 128], BF16, kind="ExternalInput").ap()
    ident = nc.dram_tensor("ident", [128, 128], BF16, kind="ExternalInput").ap()
    out = nc.dram_tensor("out", [T // N_CORES, E], F32, kind="ExternalOutput").ap()

    with tile.TileContext(nc) as tc:
        with (
            tc.tile_pool(name="consts", bufs=1) as consts,
            tc.tile_pool(name="state", bufs=1) as state,
            tc.tile_pool(name="ep", bufs=4) as ep,
            tc.tile_pool(name="rp", bufs=2) as rp,
            tc.tile_pool(name="bcp", bufs=2) as bcp,
            tc.tile_pool(name="op", bufs=4) as op,
            tc.tile_pool(name="ps_s", bufs=2, space="PSUM") as ps_s,
            tc.tile_pool(name="ps_c", bufs=3, space="PSUM") as ps_c,
            tc.tile_pool(name="ps_t", bufs=1, space="PSUM") as ps_t,
            tc.tile_pool(name="dram", bufs=1, space="DRAM") as dram,
        ):
            # ---- warm-up collective: absorbs the first-AllToAll setup cost
            # while the DMA engines are still loading x ----------------------
            wu_s = consts.tile([128, 16], BF16)
            nc.vector.memset(wu_s[:], 0.0)
            wu_in = dram.tile([N_CORES, 16, 16], BF16, tag="wu_in", name="wu_in")
            wu_out = dram.tile([N_CORES, 16, 16], BF16, tag="wu_out",
                               name="wu_out")
            nc.sync.dma_start(out=wu_in[:], in_=wu_s[:])
            nc.gpsimd.collective_compute(
                "AllToAll",
                mybir.AluOpType.bypass,
                replica_groups=[list(range(N_CORES))],
                ins=[wu_in.opt()],
                outs=[wu_out.opt()],
            )

            # ---- constants (one DMA per tensor: the per-DMA issue cost on
            # the Sync queue is ~0.6 us, so merged transfers start compute
            # much earlier) -------------------------------------------------
            def chunked(dram_ap, cols):
                # DRAM [E, cols] viewed as [p, kc, cols]: row kc*128+p
                return bass.AP(tensor=dram_ap.tensor, offset=dram_ap.offset,
                               ap=[[cols, 128], [128 * cols, NKC], [1, cols]])

            wq_sb = consts.tile([128, NKC, DPC], BF16)
            wk_sb = consts.tile([128, NKC, DPC], BF16)
            wv_sb = consts.tile([128, NKC, DPC], BF16)
            nc.sync.dma_start(out=wq_sb[:], in_=chunked(wqT, DPC))
            nc.sync.dma_start(out=wk_sb[:], in_=chunked(wkT, DPC))
            nc.sync.dma_start(out=wv_sb[:], in_=chunked(wvT, DPC))
            bq_sb = consts.tile([128, 1], F32)
            bk_sb = consts.tile([128, 1], F32)
            bv_sb = consts.tile([128, 1], F32)
            nc.sync.dma_start(out=bq_sb[:], in_=bq[:])
            nc.sync.dma_start(out=bk_sb[:], in_=bk[:])
            nc.sync.dma_start(out=bv_sb[:], in_=bv[:])
            bo_bc = consts.tile([128, E], F32)
            nc.sync.dma_start(
                out=bo_bc[:],
                in_=bass.AP(tensor=bo.tensor, offset=bo.offset,
                            ap=[[0, 128], [1, E]]),
            )
            tri_sb = consts.tile([128, 128], BF16)
            nc.sync.dma_start(out=tri_sb[:], in_=tri[:])
            id_sb = consts.tile([128, 128], BF16)
            nc.sync.dma_start(out=id_sb[:], in_=ident[:])

            # ---- x, one tile per 512-token group so projection of group t
            # only waits for its own 1 MiB ----------------------------------
            x_t = []
            for tt in range(NTT):
                xt = state.tile([128, NKC, 512], BF16, name=f"x{tt}")
                nc.sync.dma_start(
                    out=xt[:],
                    in_=bass.AP(tensor=xT.tensor, offset=xT.offset + tt * 512,
                                ap=[[T, 128], [128 * T, NKC], [1, 512]]))
                x_t.append(xt)
            wo_sb = consts.tile([128, NKC, E], BF16)
            nc.sync.dma_start(out=wo_sb[:], in_=chunked(woT, E))

            # ---- persistent activations -----------------------------------
            qT_sb = state.tile([128, T], BF16)   # [2-head dims, tokens]
            kT_sb = state.tile([128, T], BF16)
            vT_sb = state.tile([128, T], BF16)
            # per 128-token block: [64 v-dims, ones] per head -> the AV
            # matmul's 65-column stationary operand; the ones column makes
            # PSUM row 64 the softmax denominator (row 64 is 32-aligned, so
            # every DVE/broadcast access below starts at partition 0 or 64).
            vN_sb = state.tile([128, T // 128, 130], BF16)
            # normalized ctx^T: row p, plane h = head dim p of head h
            ctx2_sb = state.tile([64, 2, T], BF16)

            nc.vector.memset(vN_sb[:, :, 64:65], 1.0)
            nc.vector.memset(vN_sb[:, :, 129:130], 1.0)

            # ---- stage builders -------------------------------------------
            def emit_proj(tt):
                ts = slice(tt * 512, (tt + 1) * 512)
                ps_qk = ps_s.tile([128, 2, 512], F32, tag="s", name="ps_qk")
                for kc in range(NKC):
                    nc.tensor.matmul(ps_qk[:, 0, :], wq_sb[:, kc, :],
                                     x_t[tt][:, kc, :],
                                     start=(kc == 0), stop=(kc == NKC - 1),
                                     skip_group_check=True)
                for kc in range(NKC):
                    nc.tensor.matmul(ps_qk[:, 1, :], wk_sb[:, kc, :],
                                     x_t[tt][:, kc, :],
                                     start=(kc == 0), stop=(kc == NKC - 1),
                                     skip_group_check=True)
                ps_v = ps_s.tile([128, 2, 512], F32, tag="s", name="ps_v")
                for kc in range(NKC):
                    nc.tensor.matmul(ps_v[:, 0, :], wv_sb[:, kc, :],
                                     x_t[tt][:, kc, :],
                                     start=(kc == 0), stop=(kc == NKC - 1),
                                     skip_group_check=True)
                nc.vector.tensor_scalar_add(qT_sb[:, ts], ps_qk[:, 0, :],
                                            bq_sb[:])
                nc.vector.tensor_scalar_add(kT_sb[:, ts], ps_qk[:, 1, :],
                                            bk_sb[:])
                nc.vector.tensor_scalar_add(vT_sb[:, ts], ps_v[:, 0, :],
                                            bv_sb[:])
                for tb in range(tt * 4, tt * 4 + 4):
                    tp_ps = ps_t.tile([128, 128], BF16, tag="tp", name="tp_ps")
                    nc.tensor.transpose(
                        tp_ps[:], vT_sb[:, tb * 128:(tb + 1) * 128], id_sb[:])
                    nc.vector.tensor_copy(vN_sb[:, tb, 0:64], tp_ps[:, 0:64])
                    nc.vector.tensor_copy(vN_sb[:, tb, 65:129],
                                          tp_ps[:, 64:128])

            def emit_attn(b, qt):
                t0 = b * S
                q0 = t0 + qt * 512
                nkb = 4 * qt + 4

                def emit_scores(kb):
                    c0 = max(kb - 4 * qt, 0) * 128
                    s = ps_s.tile([128, 2, 512], F32, tag="s", name="s_ps")
                    for h in range(2):
                        d0 = h * 64
                        nc.tensor.matmul(
                            s[:, h, c0:512],
                            kT_sb[d0:d0 + 64,
                                  t0 + kb * 128:t0 + (kb + 1) * 128],
                            qT_sb[d0:d0 + 64, q0 + c0:q0 + 512],
                            start=True, stop=True, skip_group_check=True)
                    return s

                s_tiles = {0: emit_scores(0)}
                cn = [ps_c.tile([128, 512], F32, tag="cn", name=f"cn{h}")
                      for h in range(2)]
                for kb in range(nkb):
                    m = kb - 4 * qt
                    c0 = max(m, 0) * 128
                    if kb + 1 < nkb:
                        s_tiles[kb + 1] = emit_scores(kb + 1)
                    s = s_tiles.pop(kb)
                    e = ep.tile([128, 2, 512], BF16, tag="e", name="e_sb")
                    nc.scalar.activation(e[:, :, c0:512], s[:, :, c0:512],
                                         AFT.Exp, scale=0.125)
                    if m >= 0:  # triangular block on the diagonal
                        for h in range(2):
                            nc.vector.tensor_mul(e[:, h, c0:c0 + 128],
                                                 e[:, h, c0:c0 + 128],
                                                 tri_sb[:])
                    if debug_taps and b == 0 and qt == 0 and kb == 0:
                        nc.sync.dma_start(out=dbg_e[:], in_=e[:])
                    for h in range(2):
                        nc.tensor.matmul(
                            cn[h][0:65, c0:512],
                            vN_sb[:, b * SB + kb, 65 * h:65 * h + 65],
                            e[:, h, c0:512],
                            start=(kb == 0), stop=(kb == nkb - 1),
                            skip_group_check=True)

                # softmax denominator is PSUM row 64; 1/den on the DVE, then
                # broadcast down the partitions on GpSimd.  The ctx multiply
                # itself is emitted one section later (emit_norm).
                # 1/den on the DVE (fast approx), then broadcast down the
                # partitions by bouncing the row through DRAM (stride-0
                # partition read on the way back) — keeps the GpSimd queue
                # free for collective triggers, which block it until the
                # collective completes.
                rr = rp.tile([128, 2, 512], F32, tag="rr", name="rr")
                bc = bcp.tile([64, 2, 512], F32, tag="bc", name="bc")
                for h in range(2):
                    nc.vector.reciprocal(rr[64:65, h, :],
                                         cn[h][64:65, :])
                rb = dram.tile([2, 512], F32, tag="rb", name="rb", bufs=4)
                nc.sync.dma_start(out=rb[:], in_=rr[64:65, :, :])
                rb0 = rb[0]
                nc.sync.dma_start(
                    out=bc[:],
                    in_=bass.AP(tensor=rb0.tensor, offset=rb0.offset,
                                ap=[[0, 64], [512, 2], [1, 512]]))
                if debug_taps and b == 0 and qt == 0:
                    cn_cp = rp.tile([128, 2, 512], F32, tag="cncp",
                                    name="cncp")
                    nc.vector.memset(cn_cp[:], 0.0)
                    for h in range(2):
                        nc.vector.tensor_copy(cn_cp[0:65, h, :],
                                              cn[h][0:65, :])
                    nc.sync.dma_start(out=dbg_cn[:], in_=cn_cp[:])
                    nc.sync.dma_start(out=dbg_bc[:], in_=bc[:])
                    nc.sync.dma_start(out=dbg_rr[:], in_=rr[64:65, :, :])
                return cn, bc

            def emit_norm(b, qt, cn, bc):
                q0 = b * S + qt * 512
                for h in range(2):
                    nc.vector.tensor_mul(ctx2_sb[:, h, q0:q0 + 512],
                                         cn[h][0:64, :], bc[:, h, :])

            def emit_half_a2a(b, hf):
                base = b * S + hf * (S // 2)
                ctxd = dram.tile([N_CORES, 128, PH], BF16, tag="ctxd",
                                 name="ctxd", bufs=4)
                for j in range(N_CORES):
                    dst = ctxd[j]
                    nc.sync.dma_start(
                        out=bass.AP(tensor=dst.tensor, offset=dst.offset,
                                    ap=[[PH, 64], [64 * PH, 2], [1, PH]]),
                        in_=ctx2_sb[:, :, base + j * PH:base + (j + 1) * PH])
                recv = dram.tile([N_CORES, 128, PH], BF16, tag="recv",
                                 name="recv", bufs=4)
                nc.gpsimd.collective_compute(
                    "AllToAll",
                    mybir.AluOpType.bypass,
                    replica_groups=[list(range(N_CORES))],
                    ins=[ctxd.opt()],
                    outs=[recv.opt()],
                )
                return recv

            def emit_half_proj(b, hf, recv):
                cg_sb = op.tile([128, NKC, PH], BF16, tag="cg_sb", name="cg_sb",
                                bufs=2)
                # one DMA: DRAM [j][r][c] -> SBUF [r][j][c]
                r0ap = recv[0]
                nc.sync.dma_start(
                    out=cg_sb[:],
                    in_=bass.AP(tensor=r0ap.tensor, offset=r0ap.offset,
                                ap=[[PH, 128], [128 * PH, N_CORES], [1, PH]]))
                if debug_taps and b == 0 and hf == 0:
                    nc.sync.dma_start(out=dbg_cg[:], in_=cg_sb[:])
                o_sb = op.tile([PH, E], F32, tag="o_sb", name="o_sb")
                for et in range(2):
                    ps = ps_s.tile([128, 2, 512], F32, tag="s", name="c_ps")
                    for kc in range(NKC):
                        nc.tensor.matmul(
                            ps[0:PH, 0, :],
                            cg_sb[:, kc, :],
                            wo_sb[:, kc, et * 512:(et + 1) * 512],
                            start=(kc == 0), stop=(kc == NKC - 1),
                            skip_group_check=True)
                    nc.vector.tensor_add(
                        o_sb[:, et * 512:(et + 1) * 512], ps[0:PH, 0, :],
                        bo_bc[0:PH, et * 512:(et + 1) * 512])
                r0 = (b * 2 + hf) * PH
                nc.sync.dma_start(out=out[r0:r0 + PH, :], in_=o_sb[:])

            # ---- interleaved schedule -------------------------------------
            # proj(t) | attn(b, qt) sections alternate so the PE never waits
            # on DMA and the ACT-bound attention phase overlaps projection
            # matmuls.  A2A of each half-batch issues as soon as its ctx is
            # done; its Wo projection runs one section later.
            pending = []   # (b, hf, recv) with A2A issued, projection not
            norm_q = []    # (b, qt, cn, bc) with reciprocal done, mul not

            def flush_norm():
                while norm_q:
                    emit_norm(*norm_q.pop(0))

            for b in range(B):
                for qt in range(4):
                    emit_proj(b * 4 + qt)
                    flush_norm()
                    norm_q.append((b, qt) + emit_attn(b, qt))
                    if qt in (1, 3):
                        flush_norm()
                        # the Wo projection of the half-batch whose A2A was
                        # issued two sections ago: by now the collective is
                        # long done, so its matmuls never park the PE queue
                        if pending:
                            emit_half_proj(*pending.pop(0))
                        pending.append((b, qt // 2,
                                        emit_half_a2a(b, qt // 2)))
            flush_norm()
            while pending:
                emit_half_proj(*pending.pop(0))

            if debug_taps:
                nc.sync.dma_start(out=dbg_qk[:, 0, :], in_=qT_sb[:])
                nc.sync.dma_start(out=dbg_qk[:, 1, :], in_=kT_sb[:])
                nc.sync.dma_start(out=dbg_vn[:], in_=vN_sb[:])
                nc.sync.dma_start(out=dbg_ctx[:], in_=ctx2_sb[:])

    nc.compile()
    return nc


_NC = None


def _get_program():
    global _NC
    if _NC is None:
        _NC = build_program()
    return _NC


def _bf(a):
    return np.ascontiguousarray(a).astype(ml_dtypes.bfloat16)


def kernel(x, Wq, bq, Wk, bk, Wv, bv, Wo, bo, _trace=False, _trace_kwargs=None):
    x = np.asarray(x, np.float32)
    Wq, Wk, Wv, Wo = (np.asarray(w, np.float32) for w in (Wq, Wk, Wv, Wo))
    bq, bk, bv, bo = (np.asarray(v, np.float32) for v in (bq, bk, bv, bo))

    xT = _bf(x.reshape(T, E).T)
    i = np.arange(128)
    tri = _bf((i[:, None] <= i[None, :]).astype(np.float32))
    ident = _bf(np.eye(128, dtype=np.float32))

    in_maps = []
    for c in range(N_CORES):
        sl = slice(c * DPC, (c + 1) * DPC)
        in_maps.append({
            "xT": xT,
            "wqT": _bf(Wq[sl, :].T),
            "wkT": _bf(Wk[sl, :].T),
            "wvT": _bf(Wv[sl, :].T),
            "woT": _bf(Wo.T),
            "bq": bq[sl].reshape(DPC, 1).copy(),
            "bk": bk[sl].reshape(DPC, 1).copy(),
            "bv": bv[sl].reshape(DPC, 1).copy(),
            "bo": bo,
            "tri": tri,
            "ident": ident,
        })

    nc = _get_program()
    res = run_bass_kernel_spmd(nc, in_maps, list(range(N_CORES)),
                               trace=_trace, **(_trace_kwargs or {}))
    # out[c] rows are [batch, half, 128]: row (b, hf, r) holds global
    # token b*2048 + hf*1024 + c*128 + r.
    stacked = np.stack([res.results[i]["out"].reshape(B, 2, 128, E)
                        for i in range(N_CORES)], axis=2)
    full = stacked.reshape(T, E)
    if _trace:
        return full.reshape(B, S, E), res
    return full.reshape(B, S, E)


# revision 26
# speedup vs baseline: 1.1435x; 1.1435x over previous
"""Multi-head attention (B=2, S=2048, H=16, D=64) on 8 Trainium2 NeuronCores.

Head-parallel tensor parallelism: core c owns heads {2c, 2c+1} (a 128-dim
slice of the model dim): column-parallel QKV projections and local causal
attention for its 2 heads, then an AllToAll of bf16 context vectors (one
half-batch at a time, pipelined behind attention) and a full-width Wo
projection for this core's own disjoint 128-token output slices.

Shaped by trace measurements on this part:

* x loads in 8 per-token-tile DMAs and the QKV projection of tile t is
  interleaved with attention on query group t-1, so the PE starts ~20 us
  earlier than with a monolithic x load, and ACT-bound attention stretches
  overlap projection matmuls.
* Attention-times-V keeps V plus a trailing ones column as the 65-column
  stationary operand and streams the exp tile (one matmul per key block,
  N<=512): context comes out already transposed ([dims, tokens]) and the
  softmax denominator lands on PSUM partition 64.
* Scores use tile_position row pairs: each head is a K=64 matmul on its own
  row-group half of the PE array, so the two heads' score matmuls run
  concurrently (no zero-padded K=128 operands).
* exp is one ACT instruction per key block covering both heads.
* Softmax normalization happens on the *receiving* core: the AllToAll
  payload is 130 rows per peer (65 per head: 64 unnormalized ctx dims plus
  the denominator row), so the 16 denominator rows stack on the partition
  axis at the receiver where one 16-lane DVE reciprocal + a DRAM-bounced
  stride-0 broadcast + one fused multiply normalize the gathered ctx.
  (DVE reciprocal is ~8 cycles/element/lane, so sender-side row-wise
  reciprocals were 3.3 us each; gpsimd partition_broadcast and the custom
  reciprocal_approx_fast DVE op both produce wrong results on hardware.)
* Collective triggers block the GpSimd queue until the collective
  completes, so nothing else is ever placed on GpSimd, and each half-batch
  Wo projection is emitted two sections after its AllToAll was issued.
* A tiny warm-up AllToAll is issued during the load phase so the first real
  collective doesn't pay the ~23 us first-call setup on the critical path.
"""

import sys

sys.path.insert(0, "/opt/trn_rl_repo")

import ml_dtypes
import numpy as np

import concourse.bass as bass
import concourse.tile as tile
from concourse import bacc, mybir
from concourse.bass_utils import run_bass_kernel_spmd

N_CORES = 8
B, S, H, D = 2, 2048, 16, 64
E = H * D            # 1024
T = B * S            # 4096 tokens
DPC = 128            # dims (2 heads) per core
NKC = E // 128       # 8 contraction chunks for the projections
NTT = T // 512       # 8 token tiles of 512
SB = S // 128        # 16 key blocks per batch
PH = S // 2 // N_CORES  # 128 tokens per core per half-batch
CR = 130             # a2a chunk rows: 2 x (64 ctx dims + den)

F32 = mybir.dt.float32
BF16 = mybir.dt.bfloat16
AFT = mybir.ActivationFunctionType


def build_program():
    nc = bacc.Bacc("TRN2", target_bir_lowering=False, debug=False,
                   num_devices=N_CORES)

    xT = nc.dram_tensor("xT", [E, T], BF16, kind="ExternalInput").ap()
    wqT = nc.dram_tensor("wqT", [E, DPC], BF16, kind="ExternalInput").ap()
    wkT = nc.dram_tensor("wkT", [E, DPC], BF16, kind="ExternalInput").ap()
    wvT = nc.dram_tensor("wvT", [E, DPC], BF16, kind="ExternalInput").ap()
    woT = nc.dram_tensor("woT", [E, E], BF16, kind="ExternalInput").ap()
    bq = nc.dram_tensor("bq", [DPC, 1], F32, kind="ExternalInput").ap()
    bk = nc.dram_tensor("bk", [DPC, 1], F32, kind="ExternalInput").ap()
    bv = nc.dram_tensor("bv", [DPC, 1], F32, kind="ExternalInput").ap()
    bo = nc.dram_tensor("bo", [E], F32, kind="ExternalInput").ap()
    # single 128x128 lower-triangular (k_local <= q_local) mask
    tri = nc.dram_tensor("tri", [128, 128], BF16, kind="ExternalInput").ap()
    ident = nc.dram_tensor("ident", [128, 128], BF16, kind="ExternalInput").ap()
    out = nc.dram_tensor("out", [T // N_CORES, E], F32, kind="ExternalOutput").ap()

    with tile.TileContext(nc) as tc:
        with (
            tc.tile_pool(name="consts", bufs=1) as consts,
            tc.tile_pool(name="state", bufs=1) as state,
            tc.tile_pool(name="ep", bufs=4) as ep,
            tc.tile_pool(name="op", bufs=4) as op,
            tc.tile_pool(name="ps_s", bufs=2, space="PSUM") as ps_s,
            tc.tile_pool(name="ps_c", bufs=3, space="PSUM") as ps_c,
            tc.tile_pool(name="ps_t", bufs=1, space="PSUM") as ps_t,
            tc.tile_pool(name="dram", bufs=1, space="DRAM") as dram,
        ):
            # ---- warm-up collective: absorbs the first-AllToAll setup cost
            # while the DMA engines are still loading x ----------------------
            wu_s = consts.tile([128, 16], BF16)
            nc.vector.memset(wu_s[:], 0.0)
            wu_in = dram.tile([N_CORES, 16, 16], BF16, tag="wu_in", name="wu_in")
            wu_out = dram.tile([N_CORES, 16, 16], BF16, tag="wu_out",
                               name="wu_out")
            nc.sync.dma_start(out=wu_in[:], in_=wu_s[:])
            nc.gpsimd.collective_compute(
                "AllToAll",
                mybir.AluOpType.bypass,
                replica_groups=[list(range(N_CORES))],
                ins=[wu_in.opt()],
                outs=[wu_out.opt()],
            )

            # ---- constants (one DMA per tensor) ---------------------------
            def chunked(dram_ap, cols):
                # DRAM [E, cols] viewed as [p, kc, cols]: row kc*128+p
                return bass.AP(tensor=dram_ap.tensor, offset=dram_ap.offset,
                               ap=[[cols, 128], [128 * cols, NKC], [1, cols]])

            wq_sb = consts.tile([128, NKC, DPC], BF16)
            wk_sb = consts.tile([128, NKC, DPC], BF16)
            wv_sb = consts.tile([128, NKC, DPC], BF16)
            nc.sync.dma_start(out=wq_sb[:], in_=chunked(wqT, DPC))
            nc.sync.dma_start(out=wk_sb[:], in_=chunked(wkT, DPC))
            nc.sync.dma_start(out=wv_sb[:], in_=chunked(wvT, DPC))
            bq_sb = consts.tile([128, 1], F32)
            bk_sb = consts.tile([128, 1], F32)
            bv_sb = consts.tile([128, 1], F32)
            nc.sync.dma_start(out=bq_sb[:], in_=bq[:])
            nc.sync.dma_start(out=bk_sb[:], in_=bk[:])
            nc.sync.dma_start(out=bv_sb[:], in_=bv[:])
            bo_bc = consts.tile([128, E], F32)
            nc.sync.dma_start(
                out=bo_bc[:],
                in_=bass.AP(tensor=bo.tensor, offset=bo.offset,
                            ap=[[0, 128], [1, E]]),
            )
            tri_sb = consts.tile([128, 128], BF16)
            nc.sync.dma_start(out=tri_sb[:], in_=tri[:])
            id_sb = consts.tile([128, 128], BF16)
            nc.sync.dma_start(out=id_sb[:], in_=ident[:])

            # ---- x, one tile per 512-token group --------------------------
            x_t = []
            for tt in range(NTT):
                xt = state.tile([128, NKC, 512], BF16, name=f"x{tt}")
                nc.sync.dma_start(
                    out=xt[:],
                    in_=bass.AP(tensor=xT.tensor, offset=xT.offset + tt * 512,
                                ap=[[T, 128], [128 * T, NKC], [1, 512]]))
                x_t.append(xt)
            wo_sb = consts.tile([128, NKC, E], BF16)
            nc.sync.dma_start(out=wo_sb[:], in_=chunked(woT, E))

            # ---- persistent activations -----------------------------------
            qT_sb = state.tile([128, T], BF16)   # [2-head dims, tokens]
            kT_sb = state.tile([128, T], BF16)
            vT_sb = state.tile([128, T], BF16)
            # per 128-token block: [64 v-dims, ones] per head -> the AV
            # matmul's 65-column stationary operand; the ones column makes
            # PSUM row 64 the softmax denominator.
            vN_sb = state.tile([128, T // 128, 130], BF16)
            # unnormalized ctx^T + den: rows 0-63 ctx dims, row 64 den
            ctx2_sb = state.tile([65, 2, T], BF16)

            nc.vector.memset(vN_sb[:, :, 64:65], 1.0)
            nc.vector.memset(vN_sb[:, :, 129:130], 1.0)

            # ---- stage builders -------------------------------------------
            def emit_proj(tt):
                ts = slice(tt * 512, (tt + 1) * 512)
                ps_qk = ps_s.tile([128, 2, 512], F32, tag="s", name="ps_qk")
                for kc in range(NKC):
                    nc.tensor.matmul(ps_qk[:, 0, :], wq_sb[:, kc, :],
                                     x_t[tt][:, kc, :],
                                     start=(kc == 0), stop=(kc == NKC - 1),
                                     skip_group_check=True)
                for kc in range(NKC):
                    nc.tensor.matmul(ps_qk[:, 1, :], wk_sb[:, kc, :],
                                     x_t[tt][:, kc, :],
                                     start=(kc == 0), stop=(kc == NKC - 1),
                                     skip_group_check=True)
                ps_v = ps_s.tile([128, 2, 512], F32, tag="s", name="ps_v")
                for kc in range(NKC):
                    nc.tensor.matmul(ps_v[:, 0, :], wv_sb[:, kc, :],
                                     x_t[tt][:, kc, :],
                                     start=(kc == 0), stop=(kc == NKC - 1),
                                     skip_group_check=True)
                nc.vector.tensor_scalar_add(qT_sb[:, ts], ps_qk[:, 0, :],
                                            bq_sb[:])
                nc.vector.tensor_scalar_add(kT_sb[:, ts], ps_qk[:, 1, :],
                                            bk_sb[:])
                nc.vector.tensor_scalar_add(vT_sb[:, ts], ps_v[:, 0, :],
                                            bv_sb[:])
                for tb in range(tt * 4, tt * 4 + 4):
                    tp_ps = ps_t.tile([128, 128], BF16, tag="tp", name="tp_ps")
                    nc.tensor.transpose(
                        tp_ps[:], vT_sb[:, tb * 128:(tb + 1) * 128], id_sb[:])
                    nc.vector.tensor_copy(vN_sb[:, tb, 0:64], tp_ps[:, 0:64])
                    nc.vector.tensor_copy(vN_sb[:, tb, 65:129],
                                          tp_ps[:, 64:128])

            def emit_attn(b, qt):
                t0 = b * S
                q0 = t0 + qt * 512
                nkb = 4 * qt + 4

                def emit_scores(kb):
                    c0 = max(kb - 4 * qt, 0) * 128
                    s = ps_s.tile([128, 2, 512], F32, tag="s", name="s_ps")
                    for h in range(2):
                        d0 = h * 64
                        nc.tensor.matmul(
                            s[:, h, c0:512],
                            kT_sb[d0:d0 + 64,
                                  t0 + kb * 128:t0 + (kb + 1) * 128],
                            qT_sb[d0:d0 + 64, q0 + c0:q0 + 512],
                            start=True, stop=True, skip_group_check=True)
                    return s

                s_tiles = {0: emit_scores(0)}
                cn = [ps_c.tile([128, 512], F32, tag="cn", name=f"cn{h}")
                      for h in range(2)]
                for kb in range(nkb):
                    m = kb - 4 * qt
                    c0 = max(m, 0) * 128
                    if kb + 1 < nkb:
                        s_tiles[kb + 1] = emit_scores(kb + 1)
                    s = s_tiles.pop(kb)
                    e = ep.tile([128, 2, 512], BF16, tag="e", name="e_sb")
                    nc.scalar.activation(e[:, :, c0:512], s[:, :, c0:512],
                                         AFT.Exp, scale=0.125)
                    if m >= 0:  # triangular block on the diagonal
                        for h in range(2):
                            nc.vector.tensor_mul(e[:, h, c0:c0 + 128],
                                                 e[:, h, c0:c0 + 128],
                                                 tri_sb[:])
                    for h in range(2):
                        nc.tensor.matmul(
                            cn[h][0:65, c0:512],
                            vN_sb[:, b * SB + kb, 65 * h:65 * h + 65],
                            e[:, h, c0:512],
                            start=(kb == 0), stop=(kb == nkb - 1),
                            skip_group_check=True)

                # stage unnormalized ctx + den rows for the AllToAll
                for h in range(2):
                    nc.vector.tensor_copy(ctx2_sb[:, h, q0:q0 + 512],
                                          cn[h][0:65, :])

            def emit_half_a2a(b, hf):
                base = b * S + hf * (S // 2)
                ctxd = dram.tile([N_CORES, CR, PH], BF16, tag="ctxd",
                                 name="ctxd", bufs=4)
                for j in range(N_CORES):
                    dst = ctxd[j]
                    nc.sync.dma_start(
                        out=bass.AP(tensor=dst.tensor, offset=dst.offset,
                                    ap=[[PH, 65], [65 * PH, 2], [1, PH]]),
                        in_=ctx2_sb[:, :, base + j * PH:base + (j + 1) * PH])
                recv = dram.tile([N_CORES, CR, PH], BF16, tag="recv",
                                 name="recv", bufs=4)
                nc.gpsimd.collective_compute(
                    "AllToAll",
                    mybir.AluOpType.bypass,
                    replica_groups=[list(range(N_CORES))],
                    ins=[ctxd.opt()],
                    outs=[recv.opt()],
                )
                return recv

            def emit_half_proj(b, hf, recv):
                r0 = recv[0]
                cg_sb = op.tile([128, NKC, PH], BF16, tag="cg_sb", name="cg_sb",
                                bufs=2)
                for h in range(2):
                    nc.sync.dma_start(
                        out=cg_sb[h * 64:(h + 1) * 64, :, :],
                        in_=bass.AP(tensor=r0.tensor,
                                    offset=r0.offset + h * 65 * PH,
                                    ap=[[PH, 64], [CR * PH, N_CORES],
                                        [1, PH]]))
                # 16 denominator rows stacked on partitions: p = 2*j + h
                den16 = op.tile([16, PH], BF16, tag="den16", name="den16",
                                bufs=2)
                nc.sync.dma_start(
                    out=den16[:],
                    in_=bass.AP(tensor=r0.tensor, offset=r0.offset + 64 * PH,
                                ap=[[CR * PH, N_CORES], [65 * PH, 2],
                                    [1, PH]]))
                r16 = op.tile([16, PH], F32, tag="r16", name="r16", bufs=2)
                nc.vector.reciprocal(r16[:], den16[:])
                rd = dram.tile([16, PH], F32, tag="rd", name="rd", bufs=4)
                nc.sync.dma_start(out=rd[:], in_=r16[:])
                rmap = op.tile([128, NKC, PH], F32, tag="rmap", name="rmap",
                               bufs=2)
                rd0 = rd[0]
                for h in range(2):
                    nc.sync.dma_start(
                        out=rmap[h * 64:(h + 1) * 64, :, :],
                        in_=bass.AP(tensor=rd0.tensor,
                                    offset=rd0.offset + h * PH,
                                    ap=[[0, 64], [2 * PH, N_CORES], [1, PH]]))
                nc.vector.tensor_mul(cg_sb[:], cg_sb[:], rmap[:])
                o_sb = op.tile([PH, E], F32, tag="o_sb", name="o_sb")
                for et in range(2):
                    ps = ps_s.tile([128, 2, 512], F32, tag="s", name="c_ps")
                    for kc in range(NKC):
                        nc.tensor.matmul(
                            ps[0:PH, 0, :],
                            cg_sb[:, kc, :],
                            wo_sb[:, kc, et * 512:(et + 1) * 512],
                            start=(kc == 0), stop=(kc == NKC - 1),
                            skip_group_check=True)
                    nc.vector.tensor_add(
                        o_sb[:, et * 512:(et + 1) * 512], ps[0:PH, 0, :],
                        bo_bc[0:PH, et * 512:(et + 1) * 512])
                r0w = (b * 2 + hf) * PH
                nc.sync.dma_start(out=out[r0w:r0w + PH, :], in_=o_sb[:])

            # ---- interleaved schedule -------------------------------------
            pending = []   # (b, hf, recv) with A2A issued, projection not

            for b in range(B):
                for qt in range(4):
                    emit_proj(b * 4 + qt)
                    emit_attn(b, qt)
                    if qt in (1, 3):
                        if pending:
                            emit_half_proj(*pending.pop(0))
                        pending.append((b, qt // 2,
                                        emit_half_a2a(b, qt // 2)))
            while pending:
                emit_half_proj(*pending.pop(0))

    nc.compile()
    return nc


_NC = None


def _get_program():
    global _NC
    if _NC is None:
        _NC = build_program()
    return _NC


def _bf(a):
    return np.ascontiguousarray(a).astype(ml_dtypes.bfloat16)


def kernel(x, Wq, bq, Wk, bk, Wv, bv, Wo, bo, _trace=False, _trace_kwargs=None):
    x = np.asarray(x, np.float32)
    Wq, Wk, Wv, Wo = (np.asarray(w, np.float32) for w in (Wq, Wk, Wv, Wo))
    bq, bk, bv, bo = (np.asarray(v, np.float32) for v in (bq, bk, bv, bo))

    xT = _bf(x.reshape(T, E).T)
    i = np.arange(128)
    tri = _bf((i[:, None] <= i[None, :]).astype(np.float32))
    ident = _bf(np.eye(128, dtype=np.float32))

    in_maps = []
    for c in range(N_CORES):
        sl = slice(c * DPC, (c + 1) * DPC)
        in_maps.append({
            "xT": xT,
            "wqT": _bf(Wq[sl, :].T),
            "wkT": _bf(Wk[sl, :].T),
            "wvT": _bf(Wv[sl, :].T),
            "woT": _bf(Wo.T),
            "bq": bq[sl].reshape(DPC, 1).copy(),
            "bk": bk[sl].reshape(DPC, 1).copy(),
            "bv": bv[sl].reshape(DPC, 1).copy(),
            "bo": bo,
            "tri": tri,
            "ident": ident,
        })

    nc = _get_program()
    res = run_bass_kernel_spmd(nc, in_maps, list(range(N_CORES)),
                               trace=_trace, **(_trace_kwargs or {}))
    # out[c] rows are [batch, half, 128]: row (b, hf, r) holds global
    # token b*2048 + hf*1024 + c*128 + r.
    stacked = np.stack([res.results[i]["out"].reshape(B, 2, 128, E)
                        for i in range(N_CORES)], axis=2)
    full = stacked.reshape(T, E)
    if _trace:
        return full.reshape(B, S, E), res
    return full.reshape(B, S, E)


# revision 28
# speedup vs baseline: 1.2313x; 1.0768x over previous
"""Multi-head attention (B=2, S=2048, H=16, D=64) on 8 Trainium2 NeuronCores.

Head-parallel tensor parallelism: core c owns heads {2c, 2c+1} (a 128-dim
slice of the model dim): column-parallel QKV projections and local causal
attention for its 2 heads, then an AllToAll of bf16 context vectors (one
half-batch at a time, pipelined behind attention) and a full-width Wo
projection for this core's own disjoint 128-token output slices.

Shaped by trace measurements on this part:

* x loads in 8 per-token-tile DMAs and the QKV projection of tile t is
  interleaved with attention on query group t-1, so the PE starts ~20 us
  earlier than with a monolithic x load, and ACT-bound attention stretches
  overlap projection matmuls.
* Attention-times-V keeps V plus a trailing ones column as the 65-column
  stationary operand and streams the exp tile (one matmul per key block,
  N<=512): context comes out already transposed ([dims, tokens]) and the
  softmax denominator lands on PSUM partition 64.
* Scores use tile_position row pairs: each head is a K=64 matmul on its own
  row-group half of the PE array, so the two heads' score matmuls run
  concurrently (no zero-padded K=128 operands).
* exp is one ACT instruction per key block covering both heads.
* Softmax normalization happens on the *receiving* core: the AllToAll
  payload is 130 rows per peer (65 per head: 64 unnormalized ctx dims plus
  the denominator row), so the 16 denominator rows stack on the partition
  axis at the receiver where one 16-lane DVE reciprocal + a DRAM-bounced
  stride-0 broadcast + one fused multiply normalize the gathered ctx.
  (DVE reciprocal is ~8 cycles/element/lane, so sender-side row-wise
  reciprocals were 3.3 us each; gpsimd partition_broadcast and the custom
  reciprocal_approx_fast DVE op both produce wrong results on hardware.)
* Collective triggers block the GpSimd queue until the collective
  completes, so nothing else is ever placed on GpSimd, and each half-batch
  Wo projection is emitted two sections after its AllToAll was issued.
* A tiny warm-up AllToAll is issued during the load phase so the first real
  collective doesn't pay the ~23 us first-call setup on the critical path.
"""

import sys

sys.path.insert(0, "/opt/trn_rl_repo")

import ml_dtypes
import numpy as np

import concourse.bass as bass
import concourse.tile as tile
from concourse import bacc, mybir
from concourse.bass_utils import run_bass_kernel_spmd

N_CORES = 8
B, S, H, D = 2, 2048, 16, 64
E = H * D            # 1024
T = B * S            # 4096 tokens
DPC = 128            # dims (2 heads) per core
NKC = E // 128       # 8 contraction chunks for the projections
NTT = T // 512       # 8 token tiles of 512
SB = S // 128        # 16 key blocks per batch
PH = S // 2 // N_CORES  # 128 tokens per core per half-batch
CR = 130             # a2a chunk rows: 2 x (64 ctx dims + den)

F32 = mybir.dt.float32
BF16 = mybir.dt.bfloat16
AFT = mybir.ActivationFunctionType


def build_program():
    nc = bacc.Bacc("TRN2", target_bir_lowering=False, debug=False,
                   num_devices=N_CORES)

    xT = nc.dram_tensor("xT", [E, T], BF16, kind="ExternalInput").ap()
    wqT = nc.dram_tensor("wqT", [E, DPC], BF16, kind="ExternalInput").ap()
    wkT = nc.dram_tensor("wkT", [E, DPC], BF16, kind="ExternalInput").ap()
    wvT = nc.dram_tensor("wvT", [E, DPC], BF16, kind="ExternalInput").ap()
    woT = nc.dram_tensor("woT", [E, E], BF16, kind="ExternalInput").ap()
    bq = nc.dram_tensor("bq", [DPC, 1], F32, kind="ExternalInput").ap()
    bk = nc.dram_tensor("bk", [DPC, 1], F32, kind="ExternalInput").ap()
    bv = nc.dram_tensor("bv", [DPC, 1], F32, kind="ExternalInput").ap()
    bo = nc.dram_tensor("bo", [E], F32, kind="ExternalInput").ap()
    # single 128x128 lower-triangular (k_local <= q_local) mask
    tri = nc.dram_tensor("tri", [128, 128], BF16, kind="ExternalInput").ap()
    ident = nc.dram_tensor("ident", [128, 128], BF16, kind="ExternalInput").ap()
    out = nc.dram_tensor("out", [T // N_CORES, E], F32, kind="ExternalOutput").ap()

    with tile.TileContext(nc) as tc:
        with (
            tc.tile_pool(name="consts", bufs=1) as consts,
            tc.tile_pool(name="state", bufs=1) as state,
            tc.tile_pool(name="ep", bufs=4) as ep,
            tc.tile_pool(name="op", bufs=4) as op,
            tc.tile_pool(name="ps_s", bufs=2, space="PSUM") as ps_s,
            tc.tile_pool(name="ps_c", bufs=3, space="PSUM") as ps_c,
            tc.tile_pool(name="ps_t", bufs=1, space="PSUM") as ps_t,
            tc.tile_pool(name="dram", bufs=1, space="DRAM") as dram,
        ):
            # ---- warm-up collective: absorbs the first-AllToAll setup cost
            # while the DMA engines are still loading x ----------------------
            wu_s = consts.tile([128, 16], BF16)
            nc.vector.memset(wu_s[:], 0.0)
            wu_in = dram.tile([N_CORES, 16, 16], BF16, tag="wu_in", name="wu_in")
            wu_out = dram.tile([N_CORES, 16, 16], BF16, tag="wu_out",
                               name="wu_out")
            nc.sync.dma_start(out=wu_in[:], in_=wu_s[:])
            nc.gpsimd.collective_compute(
                "AllToAll",
                mybir.AluOpType.bypass,
                replica_groups=[list(range(N_CORES))],
                ins=[wu_in.opt()],
                outs=[wu_out.opt()],
            )

            # ---- constants (one DMA per tensor) ---------------------------
            def chunked(dram_ap, cols):
                # DRAM [E, cols] viewed as [p, kc, cols]: row kc*128+p
                return bass.AP(tensor=dram_ap.tensor, offset=dram_ap.offset,
                               ap=[[cols, 128], [128 * cols, NKC], [1, cols]])

            wq_sb = consts.tile([128, NKC, DPC], BF16)
            wk_sb = consts.tile([128, NKC, DPC], BF16)
            wv_sb = consts.tile([128, NKC, DPC], BF16)
            nc.sync.dma_start(out=wq_sb[:], in_=chunked(wqT, DPC))
            nc.sync.dma_start(out=wk_sb[:], in_=chunked(wkT, DPC))
            nc.sync.dma_start(out=wv_sb[:], in_=chunked(wvT, DPC))
            bq_sb = consts.tile([128, 1], F32)
            bk_sb = consts.tile([128, 1], F32)
            bv_sb = consts.tile([128, 1], F32)
            nc.sync.dma_start(out=bq_sb[:], in_=bq[:])
            nc.sync.dma_start(out=bk_sb[:], in_=bk[:])
            nc.sync.dma_start(out=bv_sb[:], in_=bv[:])
            bo_bc = consts.tile([128, E], F32)
            nc.sync.dma_start(
                out=bo_bc[:],
                in_=bass.AP(tensor=bo.tensor, offset=bo.offset,
                            ap=[[0, 128], [1, E]]),
            )
            tri_sb = consts.tile([128, 128], BF16)
            nc.sync.dma_start(out=tri_sb[:], in_=tri[:])
            id_sb = consts.tile([128, 128], BF16)
            nc.sync.dma_start(out=id_sb[:], in_=ident[:])

            # ---- x, one tile per 512-token group --------------------------
            x_t = []
            for tt in range(NTT):
                xt = state.tile([128, NKC, 512], BF16, name=f"x{tt}")
                nc.sync.dma_start(
                    out=xt[:],
                    in_=bass.AP(tensor=xT.tensor, offset=xT.offset + tt * 512,
                                ap=[[T, 128], [128 * T, NKC], [1, 512]]))
                x_t.append(xt)
            wo_sb = consts.tile([128, NKC, E], BF16)
            nc.sync.dma_start(out=wo_sb[:], in_=chunked(woT, E))

            # ---- persistent activations -----------------------------------
            qT_sb = state.tile([128, T], BF16)   # [2-head dims, tokens]
            kT_sb = state.tile([128, T], BF16)
            vT_sb = state.tile([128, T], BF16)
            # per 128-token block: [64 v-dims, ones] per head -> the AV
            # matmul's 65-column stationary operand; the ones column makes
            # PSUM row 64 the softmax denominator.
            vN_sb = state.tile([128, T // 128, 130], BF16)
            # unnormalized ctx^T + den: rows 0-63 ctx dims, row 64 den
            ctx2_sb = state.tile([65, 2, T], BF16)

            nc.vector.memset(vN_sb[:, :, 64:65], 1.0)
            nc.vector.memset(vN_sb[:, :, 129:130], 1.0)

            # ---- stage builders -------------------------------------------
            def emit_proj(tt):
                ts = slice(tt * 512, (tt + 1) * 512)
                ps_qk = ps_s.tile([128, 2, 512], F32, tag="s", name="ps_qk")
                for kc in range(NKC):
                    nc.tensor.matmul(ps_qk[:, 0, :], wq_sb[:, kc, :],
                                     x_t[tt][:, kc, :],
                                     start=(kc == 0), stop=(kc == NKC - 1),
                                     skip_group_check=True)
                for kc in range(NKC):
                    nc.tensor.matmul(ps_qk[:, 1, :], wk_sb[:, kc, :],
                                     x_t[tt][:, kc, :],
                                     start=(kc == 0), stop=(kc == NKC - 1),
                                     skip_group_check=True)
                ps_v = ps_s.tile([128, 2, 512], F32, tag="s", name="ps_v")
                for kc in range(NKC):
                    nc.tensor.matmul(ps_v[:, 0, :], wv_sb[:, kc, :],
                                     x_t[tt][:, kc, :],
                                     start=(kc == 0), stop=(kc == NKC - 1),
                                     skip_group_check=True)
                nc.vector.tensor_scalar_add(qT_sb[:, ts], ps_qk[:, 0, :],
                                            bq_sb[:])
                nc.vector.tensor_scalar_add(kT_sb[:, ts], ps_qk[:, 1, :],
                                            bk_sb[:])
                nc.vector.tensor_scalar_add(vT_sb[:, ts], ps_v[:, 0, :],
                                            bv_sb[:])
                for tb in range(tt * 4, tt * 4 + 4):
                    tp_ps = ps_t.tile([128, 128], BF16, tag="tp", name="tp_ps")
                    nc.tensor.transpose(
                        tp_ps[:], vT_sb[:, tb * 128:(tb + 1) * 128], id_sb[:])
                    nc.vector.tensor_copy(vN_sb[:, tb, 0:64], tp_ps[:, 0:64])
                    nc.vector.tensor_copy(vN_sb[:, tb, 65:129],
                                          tp_ps[:, 64:128])

            def emit_attn(b, qt):
                t0 = b * S
                q0 = t0 + qt * 512
                nkb = 4 * qt + 4

                def emit_scores(kb):
                    c0 = max(kb - 4 * qt, 0) * 128
                    s = ps_s.tile([128, 2, 512], F32, tag="s", name="s_ps")
                    for h in range(2):
                        d0 = h * 64
                        nc.tensor.matmul(
                            s[:, h, c0:512],
                            kT_sb[d0:d0 + 64,
                                  t0 + kb * 128:t0 + (kb + 1) * 128],
                            qT_sb[d0:d0 + 64, q0 + c0:q0 + 512],
                            start=True, stop=True, skip_group_check=True)
                    return s

                s_tiles = {0: emit_scores(0)}
                cn = [ps_c.tile([128, 512], F32, tag="cn", name=f"cn{h}")
                      for h in range(2)]
                for kb in range(nkb):
                    m = kb - 4 * qt
                    c0 = max(m, 0) * 128
                    if kb + 1 < nkb:
                        s_tiles[kb + 1] = emit_scores(kb + 1)
                    s = s_tiles.pop(kb)
                    e = ep.tile([128, 2, 512], BF16, tag="e", name="e_sb")
                    nc.scalar.activation(e[:, :, c0:512], s[:, :, c0:512],
                                         AFT.Exp, scale=0.125)
                    if m >= 0:  # triangular block on the diagonal
                        for h in range(2):
                            nc.vector.tensor_mul(e[:, h, c0:c0 + 128],
                                                 e[:, h, c0:c0 + 128],
                                                 tri_sb[:])
                    for h in range(2):
                        nc.tensor.matmul(
                            cn[h][0:65, c0:512],
                            vN_sb[:, b * SB + kb, 65 * h:65 * h + 65],
                            e[:, h, c0:512],
                            start=(kb == 0), stop=(kb == nkb - 1),
                            skip_group_check=True)

                # stage unnormalized ctx + den rows for the AllToAll
                for h in range(2):
                    nc.vector.tensor_copy(ctx2_sb[:, h, q0:q0 + 512],
                                          cn[h][0:65, :])

            def emit_half_a2a(b, hf):
                base = b * S + hf * (S // 2)
                ctxd = dram.tile([N_CORES, CR, PH], BF16, tag="ctxd",
                                 name="ctxd", bufs=4)
                for j in range(N_CORES):
                    dst = ctxd[j]
                    nc.sync.dma_start(
                        out=bass.AP(tensor=dst.tensor, offset=dst.offset,
                                    ap=[[PH, 65], [65 * PH, 2], [1, PH]]),
                        in_=ctx2_sb[:, :, base + j * PH:base + (j + 1) * PH])
                recv = dram.tile([N_CORES, CR, PH], BF16, tag="recv",
                                 name="recv", bufs=4)
                nc.gpsimd.collective_compute(
                    "AllToAll",
                    mybir.AluOpType.bypass,
                    replica_groups=[list(range(N_CORES))],
                    ins=[ctxd.opt()],
                    outs=[recv.opt()],
                )
                return recv

            def emit_half_recv(b, hf, recv):
                # gather + normalize the received ctx; no PE work, so the PE
                # queue never parks on this chain
                r0 = recv[0]
                cg_sb = op.tile([128, NKC, PH], BF16, tag="cg_sb", name="cg_sb",
                                bufs=2)
                for h in range(2):
                    nc.sync.dma_start(
                        out=cg_sb[h * 64:(h + 1) * 64, :, :],
                        in_=bass.AP(tensor=r0.tensor,
                                    offset=r0.offset + h * 65 * PH,
                                    ap=[[PH, 64], [CR * PH, N_CORES],
                                        [1, PH]]))
                # 16 denominator rows stacked on partitions: p = 2*j + h
                den16 = op.tile([16, PH], BF16, tag="den16", name="den16",
                                bufs=2)
                nc.sync.dma_start(
                    out=den16[:],
                    in_=bass.AP(tensor=r0.tensor, offset=r0.offset + 64 * PH,
                                ap=[[CR * PH, N_CORES], [65 * PH, 2],
                                    [1, PH]]))
                r16 = op.tile([16, PH], F32, tag="r16", name="r16", bufs=2)
                nc.vector.reciprocal(r16[:], den16[:])
                rd = dram.tile([16, PH], F32, tag="rd", name="rd", bufs=4)
                nc.sync.dma_start(out=rd[:], in_=r16[:])
                rmap = op.tile([128, NKC, PH], F32, tag="rmap", name="rmap",
                               bufs=2)
                rd0 = rd[0]
                for h in range(2):
                    nc.sync.dma_start(
                        out=rmap[h * 64:(h + 1) * 64, :, :],
                        in_=bass.AP(tensor=rd0.tensor,
                                    offset=rd0.offset + h * PH,
                                    ap=[[0, 64], [2 * PH, N_CORES], [1, PH]]))
                nc.vector.tensor_mul(cg_sb[:], cg_sb[:], rmap[:])
                return b, hf, cg_sb

            def emit_half_wo(b, hf, cg_sb):
                o_sb = op.tile([PH, E], F32, tag="o_sb", name="o_sb")
                for et in range(2):
                    ps = ps_s.tile([128, 2, 512], F32, tag="s", name="c_ps")
                    for kc in range(NKC):
                        nc.tensor.matmul(
                            ps[0:PH, 0, :],
                            cg_sb[:, kc, :],
                            wo_sb[:, kc, et * 512:(et + 1) * 512],
                            start=(kc == 0), stop=(kc == NKC - 1),
                            skip_group_check=True)
                    nc.vector.tensor_add(
                        o_sb[:, et * 512:(et + 1) * 512], ps[0:PH, 0, :],
                        bo_bc[0:PH, et * 512:(et + 1) * 512])
                r0w = (b * 2 + hf) * PH
                nc.sync.dma_start(out=out[r0w:r0w + PH, :], in_=o_sb[:])

            # ---- interleaved schedule -------------------------------------
            # Per half-batch: A2A issued at its boundary; gather+normalize one
            # boundary later (collective long done); Wo matmuls one boundary
            # after that (inputs ready the moment the PE reaches them).
            a2aq = []  # (b, hf, recv): A2A issued, recv-chain not emitted
            woq = []   # (b, hf, cg_sb): normalized, Wo matmuls not emitted

            for b in range(B):
                for qt in range(4):
                    emit_proj(b * 4 + qt)
                    emit_attn(b, qt)
                    if qt in (1, 3):
                        if woq:
                            emit_half_wo(*woq.pop(0))
                        if a2aq:
                            woq.append(emit_half_recv(*a2aq.pop(0)))
                        a2aq.append((b, qt // 2,
                                     emit_half_a2a(b, qt // 2)))
            while woq or a2aq:
                if woq:
                    emit_half_wo(*woq.pop(0))
                if a2aq:
                    woq.append(emit_half_recv(*a2aq.pop(0)))

    nc.compile()
    return nc


_NC = None


def _get_program():
    global _NC
    if _NC is None:
        _NC = build_program()
    return _NC


def _bf(a):
    return np.ascontiguousarray(a).astype(ml_dtypes.bfloat16)


def kernel(x, Wq, bq, Wk, bk, Wv, bv, Wo, bo, _trace=False, _trace_kwargs=None):
    x = np.asarray(x, np.float32)
    Wq, Wk, Wv, Wo = (np.asarray(w, np.float32) for w in (Wq, Wk, Wv, Wo))
    bq, bk, bv, bo = (np.asarray(v, np.float32) for v in (bq, bk, bv, bo))

    xT = _bf(x.reshape(T, E).T)
    i = np.arange(128)
    tri = _bf((i[:, None] <= i[None, :]).astype(np.float32))
    ident = _bf(np.eye(128, dtype=np.float32))

    in_maps = []
    for c in range(N_CORES):
        sl = slice(c * DPC, (c + 1) * DPC)
        in_maps.append({
            "xT": xT,
            "wqT": _bf(Wq[sl, :].T),
            "wkT": _bf(Wk[sl, :].T),
            "wvT": _bf(Wv[sl, :].T),
            "woT": _bf(Wo.T),
            "bq": bq[sl].reshape(DPC, 1).copy(),
            "bk": bk[sl].reshape(DPC, 1).copy(),
            "bv": bv[sl].reshape(DPC, 1).copy(),
            "bo": bo,
            "tri": tri,
            "ident": ident,
        })

    nc = _get_program()
    res = run_bass_kernel_spmd(nc, in_maps, list(range(N_CORES)),
                               trace=_trace, **(_trace_kwargs or {}))
    # out[c] rows are [batch, half, 128]: row (b, hf, r) holds global
    # token b*2048 + hf*1024 + c*128 + r.
    stacked = np.stack([res.results[i]["out"].reshape(B, 2, 128, E)
                        for i in range(N_CORES)], axis=2)
    full = stacked.reshape(T, E)
    if _trace:
        return full.reshape(B, S, E), res
    return full.reshape(B, S, E)
